# revision 2
# baseline (speedup 1.0000x reference)
"""BWGNN (Bernstein-polynomial graph conv, D=2) on 8 Trainium2 NeuronCores. v3.

Algebra as v1/v2 (two SpMMs; thetas folded into W3).  SpMM engine:

- Table is fp16 PAIRS: AllGather output [NPAD, 64] fp16; gather elements are
  two consecutive node rows (256B).  idx = pair index, split in 2 halves of
  25088 (int16 range); gathered data feeds matmul rhs directly (no convert).
- Slot layout: per (block b, half h) a run of cross-core equalized length
  L[b,h] = max_c count, edges sorted by parity (even cols then odd) inside
  the run.  Runs packed half-major per group of GRP blocks; segments padded
  to 128 slots.  One dma_gather per (group, half).
- S matrices are HOST-BUILT one-hots (val at [slot, dest-row]) streamed from
  HBM in piece order, 32 pieces per DMA; piece = chunk x run x parity
  intersection; matmul rhs column-half = parity of the piece.
- PSUM [128, GRP*64] per group; f_next = f - psum per block (DVE subtract).
"""
import math
import numpy as np

import concourse.bass as bass
import concourse.bacc as bacc
import concourse.mybir as mybir
from concourse.tile import TileContext
from concourse.masks import make_identity
from concourse import bass_utils

N = 100000
F_IN = 128
H = 64
NCLS = 2
D = 2
W = 8                   # cores
R = 12500               # real rows per core
RP = 12544              # padded rows per core (98 * 128)
NB = RP // 128          # 98 dest blocks per core
NPAD = W * RP           # 100352 padded table rows
NPAIR = NPAD // 2       # 50176 gather elements (256B each)
NH = 2                  # halves (int16 index range)
HS = NPAIR // NH        # 25088 pairs per half
GRP = 7                 # dest blocks per group (98 = 14 * 7)
NG = NB // GRP
SM_B = 32               # S matrices per streaming tile
F16 = mybir.dt.float16
F32 = mybir.dt.float32
I16 = mybir.dt.int16


def _theta2():
    P = np.polynomial.polynomial
    thetas = []
    for i in range(D + 1):
        beta = math.factorial(i) * math.factorial(D - i) / math.factorial(D + 1)
        c = P.polymul(P.polypow([0.0, 0.5], i), P.polypow([1.0, -0.5], D - i)) / beta
        c = np.pad(c, (0, D + 1 - len(c)))
        thetas.append(c.astype(np.float64))
    return thetas


def _prep_edges(adj_rows, adj_cols, adj_vals):
    """Slot layout with cross-core equalized (block, half) runs, parity-sorted.

    Returns:
      idx_wrapped[c]: [128, S/16] int16 16-wrapped 8x-replicated pair idxs
      smat[c]: [128, npieces*128] fp16 host-built one-hot S matrices
      schedule: static layout (segments, pieces)
    """
    core = adj_rows // R
    rloc = adj_rows - core * R
    blk = rloc // 128
    rowin = rloc % 128
    colp = (adj_cols // R) * RP + (adj_cols % R)     # padded table row
    pair = colp // 2
    par = colp % 2
    half = (pair >= HS).astype(np.int64)
    pidx = pair - half * HS                          # half-local pair index

    # counts per (core, block, half, parity); runs equalized at (b, h) level
    # with the parity boundary ALSO equalized (sort by parity inside run and
    # pad each parity sub-run to its own cross-core max so the boundary is
    # static).
    counts = np.zeros((W, NB, NH, 2), dtype=np.int64)
    np.add.at(counts, (core, blk, half, par), 1)
    Lp = counts.max(axis=0)                          # [NB, NH, 2]

    sub_start = np.zeros((NB, NH, 2), dtype=np.int64)
    seg_start = np.zeros((NG, NH), dtype=np.int64)
    seg_len = np.zeros((NG, NH), dtype=np.int64)
    pos = 0
    for g in range(NG):
        blocks = range(g * GRP, (g + 1) * GRP)
        for hh in range(NH):
            seg_start[g, hh] = pos
            for b in blocks:
                for pp in range(2):
                    sub_start[b, hh, pp] = pos
                    pos += Lp[b, hh, pp]
            pad = -pos % 128
            pos += pad
            seg_len[g, hh] = pos - seg_start[g, hh]
    S_slots = pos
    nchunks = S_slots // 128

    # pieces: (chunk, group, b7, parity, start, stop) in issue order.
    # start/stop are PER BANK (whole psum group tile): matmul start=True
    # clears has_written bits for the entire 2KB zero region, so only the
    # group's first piece starts and its last piece stops; blocks first
    # touched later in the group overwrite via cleared has_written bits.
    pieces = []
    piece_span = []      # (s0, s1) slot range of the piece
    for g in range(NG):
        g_first = len(pieces)
        for hh in range(NH):
            for b in range(g * GRP, (g + 1) * GRP):
                for pp in range(2):
                    s0 = int(sub_start[b, hh, pp])
                    s1 = s0 + int(Lp[b, hh, pp])
                    if s1 == s0:
                        continue
                    for t in range(s0 // 128, (s1 - 1) // 128 + 1):
                        pieces.append([t, g, b - g * GRP, pp, False, False])
                        piece_span.append((max(s0, t * 128),
                                           min(s1, (t + 1) * 128)))
        pieces[g_first][4] = True
        pieces[-1][5] = True
    npieces = len(pieces)

    # per-core slot data
    order = np.lexsort((pidx, par, half, blk, core))
    sc = core[order]
    csel = np.searchsorted(sc, np.arange(W + 1))
    idx_flat = np.zeros((W, S_slots), dtype=np.int16)
    rowv_slot = -np.ones((W, S_slots), dtype=np.int32)
    vals_slot = np.zeros((W, S_slots), dtype=np.float16)
    sb, sh, sp = blk[order], half[order], par[order]
    s_pidx, s_rowin, s_val = pidx[order], rowin[order], adj_vals[order]
    for c in range(W):
        lo, hi = csel[c], csel[c + 1]
        b_arr, h_arr, p_arr = sb[lo:hi], sh[lo:hi], sp[lo:hi]
        key = (b_arr * NH + h_arr) * 2 + p_arr
        brk = np.nonzero(np.diff(key))[0] + 1
        starts = np.concatenate([[0], brk])
        lens = np.diff(np.concatenate([starts, [hi - lo]]))
        pos_in = np.arange(hi - lo) - np.repeat(starts, lens)
        slot = sub_start[b_arr, h_arr, p_arr] + pos_in
        idx_flat[c, slot] = s_pidx[lo:hi].astype(np.int16)
        rowv_slot[c, slot] = s_rowin[lo:hi]
        vals_slot[c, slot] = s_val[lo:hi].astype(np.float16)

    idx_wrapped = []
    for c in range(W):
        a = idx_flat[c].reshape(S_slots // 16, 16).T
        idx_wrapped.append(np.tile(a, (8, 1)).copy())

    # host-built smat [128, npieces*128] fp16 per core
    smat = np.zeros((W, 128, npieces * 128), dtype=np.float16)
    for j, (s0, s1) in enumerate(piece_span):
        t = pieces[j][0]
        p0, p1 = s0 - t * 128, s1 - t * 128
        for c in range(W):
            rv = rowv_slot[c, s0:s1]
            vv = vals_slot[c, s0:s1]
            sel = rv >= 0
            pp = np.nonzero(sel)[0]
            smat[c, p0 + pp, j * 128 + rv[pp]] = vv[pp]

    schedule = dict(S_slots=S_slots, nchunks=nchunks, pieces=pieces,
                    seg_start=seg_start, seg_len=seg_len)
    return idx_wrapped, smat, schedule


def _build(schedule):
    S_slots = schedule["S_slots"]
    pieces = schedule["pieces"]
    seg_start = schedule["seg_start"]
    seg_len = schedule["seg_len"]
    npieces = len(pieces)
    gmax = int(max(sum(seg_len[g]) for g in range(NG))) // 128   # chunks/group

    nc = bacc.Bacc("TRN2")
    rg = [list(range(W))]

    xT = nc.dram_tensor("xT", [F_IN, RP], F16, kind="ExternalInput")
    w1 = nc.dram_tensor("w1", [F_IN, H], F16, kind="ExternalInput")
    w2 = nc.dram_tensor("w2", [H, H], F16, kind="ExternalInput")
    w3 = nc.dram_tensor("w3", [3 * H, H], F16, kind="ExternalInput")
    w4 = nc.dram_tensor("w4", [H, NCLS], F16, kind="ExternalInput")
    b1 = nc.dram_tensor("b1", [H, 1], F32, kind="ExternalInput")
    b2 = nc.dram_tensor("b2", [H, 1], F32, kind="ExternalInput")
    b3 = nc.dram_tensor("b3", [H, 1], F32, kind="ExternalInput")
    b4 = nc.dram_tensor("b4", [NCLS, 1], F32, kind="ExternalInput")
    idx_t = nc.dram_tensor("idx", [128, S_slots // 16], I16, kind="ExternalInput")
    smat_t = nc.dram_tensor("smat", [128, npieces * 128], F16,
                            kind="ExternalInput")
    out_t = nc.dram_tensor("out", [NCLS, RP], F32, kind="ExternalOutput")


    ag_in = [nc.dram_tensor(f"agin{i}", [RP, H], F16, kind="Internal")
             for i in range(2)]
    ag_out = [nc.dram_tensor(f"agout{i}", [NPAD, H], F16, kind="Internal",
                             addr_space="Shared") for i in range(2)]

    PCH = 448            # dense-layer column chunk (28 * 448 = 12544)

    with TileContext(nc) as tc:
        with tc.tile_pool(name="c0", bufs=1) as cpool, \
             tc.tile_pool(name="mm", bufs=3) as mpool, \
             tc.tile_pool(name="gg", bufs=2) as gpool, \
             tc.tile_pool(name="ss", bufs=3) as spool, \
             tc.tile_pool(name="ps", bufs=2, space="PSUM") as pspool, \
             tc.tile_pool(name="pb", bufs=2, space="PSUM") as pbpool, \
             tc.tile_pool(name="pg", bufs=2, space="PSUM") as pgpool:

            ident = cpool.tile([128, 128], F16)
            make_identity(nc, ident[:])

            def load_const(name, src, shape, dt):
                tile = cpool.tile(shape, dt, tag=name)
                nc.sync.dma_start(out=tile[:], in_=src)
                return tile

            w1_sb = load_const("w1", w1[:], [F_IN, H], F16)
            w2_sb = load_const("w2", w2[:], [H, H], F16)
            w3ab_sb = load_const("w3ab", w3[0:128, :], [128, H], F16)
            w3c_sb = load_const("w3c", w3[128:192, :], [H, H], F16)
            w4_sb = load_const("w4", w4[:], [H, NCLS], F16)
            b1_sb = load_const("b1", b1[:], [H, 1], F32)
            b2_sb = load_const("b2", b2[:], [H, 1], F32)
            b3_sb = load_const("b3", b3[:], [H, 1], F32)
            b4_sb = load_const("b4", b4[:], [NCLS, 1], F32)
            idx_sb = load_const("idx", idx_t[:], [128, S_slots // 16], I16)

            h1_f2 = cpool.tile([128, RP], F16)   # h1 then feat2 (fm) on p0..63
            h_cat = cpool.tile([128, RP], F16)   # feat0 p0..63, feat1 p64..127
            f0_rm = cpool.tile([128, NB * H], F16)
            f1_rm = cpool.tile([128, NB * H], F16)
            f2_rm = f0_rm     # feat0_rm dead once SpMM1's subtract ran

            # ---------- MLP1 + MLP2 (feature-major fp16) ----------
            for o in range(0, RP, PCH):
                xt = mpool.tile([F_IN, PCH], F16, tag="xin")
                nc.sync.dma_start(out=xt[:], in_=xT[:, o:o + PCH])
                pt = pspool.tile([H, PCH], F32, tag="pmlp", space="PSUM")
                nc.tensor.matmul(pt[:], lhsT=w1_sb[:], rhs=xt[:],
                                 start=True, stop=True)
                nc.scalar.activation(h1_f2[0:H, o:o + PCH], pt[:],
                                     mybir.ActivationFunctionType.Relu,
                                     bias=b1_sb[:], scale=1.0)
            for o in range(0, RP, PCH):
                pt = pspool.tile([H, PCH], F32, tag="pmlp", space="PSUM")
                nc.tensor.matmul(pt[:], lhsT=w2_sb[:], rhs=h1_f2[0:H, o:o + PCH],
                                 start=True, stop=True)
                nc.scalar.activation(h_cat[0:H, o:o + PCH], pt[:],
                                     mybir.ActivationFunctionType.Relu,
                                     bias=b2_sb[:], scale=1.0)

            # ---------- feat0 -> row-major, ship to AllGather ----------
            for b in range(NB):
                pt = pbpool.tile([128, 128], F16, tag="ptr", space="PSUM")
                nc.tensor.transpose(pt[0:128, 0:H],
                                    h_cat[0:H, b * 128:(b + 1) * 128],
                                    ident[0:H, 0:H])
                nc.vector.tensor_copy(f0_rm[:, b * H:(b + 1) * H], pt[0:128, 0:H])
            nc.sync.dma_start(
                out=ag_in[0][:].rearrange("(t p) h -> p t h", p=128),
                in_=f0_rm[:].rearrange("p (t h) -> p t h", h=H))
            nc.gpsimd.collective_compute(
                "AllGather", mybir.AluOpType.bypass, replica_groups=rg,
                ins=[ag_in[0][:]], outs=[ag_out[0][:]])

            # ---------- SpMM pass ----------
            def spmm(src, cur_rm, nxt_rm, ag_next):
                src_flat = src[:].rearrange("n h -> (n h)")
                hviews = [src_flat[hh * HS * 128:(hh + 1) * HS * 128].rearrange(
                    "(q s) -> q s", s=128) for hh in range(NH)]
                pc = 0
                for g in range(NG):
                    g16 = gpool.tile([128, gmax * 128], F16, tag="g16")
                    goff = 0
                    for hh in range(NH):
                        s0 = int(seg_start[g, hh])
                        ln = int(seg_len[g, hh])
                        lc = ln // 128
                        nc.gpsimd.dma_gather(
                            out_ap=g16[:, goff * 128:(goff + lc) * 128]
                            .rearrange("p (t e) -> p t e", e=128),
                            in_ap=hviews[hh],
                            idxs_ap=idx_sb[:, s0 // 16:(s0 + ln) // 16],
                            num_idxs=ln,
                            num_idxs_reg=ln,
                            elem_size=128,
                            single_packet=False,
                        )
                        goff += lc
                    gbase = int(seg_start[g, 0]) // 128
                    psum_g = pgpool.tile([128, GRP * H], F32, tag="pgrp",
                                         space="PSUM")
                    while pc < npieces and pieces[pc][1] == g:
                        # stream SM_B pieces of smat at a time
                        j0 = pc
                        j1 = min(j0 + SM_B, npieces)
                        while j1 > j0 and pieces[j1 - 1][1] != g:
                            j1 -= 1
                        sm = spool.tile([128, SM_B * 128], F16, tag="sm")
                        nc.sync.dma_start(
                            out=sm[:, 0:(j1 - j0) * 128],
                            in_=smat_t[:, j0 * 128:j1 * 128])
                        for j in range(j0, j1):
                            t, _, b7, pp, st, sp = pieces[j]
                            tl = t - gbase
                            nc.tensor.matmul(
                                psum_g[:, b7 * H:(b7 + 1) * H],
                                lhsT=sm[:, (j - j0) * 128:(j - j0 + 1) * 128],
                                rhs=g16[:, tl * 128 + pp * H:
                                        tl * 128 + (pp + 1) * H],
                                start=st, stop=sp)
                        pc = j1
                    for b7 in range(GRP):
                        b = g * GRP + b7
                        nc.vector.tensor_tensor(
                            out=nxt_rm[:, b * H:(b + 1) * H],
                            in0=cur_rm[:, b * H:(b + 1) * H],
                            in1=psum_g[:, b7 * H:(b7 + 1) * H],
                            op=mybir.AluOpType.subtract)
                if ag_next is not None:
                    nc.sync.dma_start(
                        out=ag_next[:].rearrange("(t p) h -> p t h", p=128),
                        in_=nxt_rm[:].rearrange("p (t h) -> p t h", h=H))

            spmm(ag_out[0], f0_rm, f1_rm, ag_in[1])
            nc.gpsimd.collective_compute(
                "AllGather", mybir.AluOpType.bypass, replica_groups=rg,
                ins=[ag_in[1][:]], outs=[ag_out[1][:]])

            # f1 -> feature-major while AllGather of f1 runs
            for b in range(NB):
                pt = pbpool.tile([128, 128], F16, tag="ptr", space="PSUM")
                nc.tensor.transpose(pt[0:H, 0:128], f1_rm[:, b * H:(b + 1) * H],
                                    ident[:])
                nc.vector.tensor_copy(h_cat[H:128, b * 128:(b + 1) * 128],
                                      pt[0:H, 0:128])

            spmm(ag_out[1], f1_rm, f2_rm, None)

            # ---------- feat2 back to feature-major ----------
            for b in range(NB):
                pt = pbpool.tile([128, 128], F16, tag="ptr", space="PSUM")
                nc.tensor.transpose(pt[0:H, 0:128], f2_rm[:, b * H:(b + 1) * H],
                                    ident[:])
                nc.vector.tensor_copy(h1_f2[0:H, b * 128:(b + 1) * 128],
                                      pt[0:H, 0:128])

            # ---------- MLP3 + MLP4 fused ----------
            for o in range(0, RP, PCH):
                pt = pspool.tile([H, PCH], F32, tag="pmlp", space="PSUM")
                nc.tensor.matmul(pt[:], lhsT=w3ab_sb[:], rhs=h_cat[:, o:o + PCH],
                                 start=True, stop=False)
                nc.tensor.matmul(pt[:], lhsT=w3c_sb[:], rhs=h1_f2[0:H, o:o + PCH],
                                 start=False, stop=True)
                h3 = mpool.tile([H, PCH], F16, tag="h3")
                nc.scalar.activation(h3[:], pt[:],
                                     mybir.ActivationFunctionType.Relu,
                                     bias=b3_sb[:], scale=1.0)
                po = pspool.tile([NCLS, PCH], F32, tag="pout", space="PSUM")
                nc.tensor.matmul(po[:], lhsT=w4_sb[:], rhs=h3[:],
                                 start=True, stop=True)
                ot = mpool.tile([NCLS, PCH], F32, tag="ot")
                nc.scalar.activation(ot[:], po[:],
                                     mybir.ActivationFunctionType.Identity,
                                     bias=b4_sb[:], scale=1.0)
                nc.sync.dma_start(out=out_t[:, o:o + PCH], in_=ot[:])

    nc.compile()
    return nc


def prepare(inputs):
    """Build (nc, in_maps) for the full input dict."""
    in_feat = np.asarray(inputs["in_feat"], dtype=np.float32)
    adj_rows = np.asarray(inputs["adj_rows"]).astype(np.int64)
    adj_cols = np.asarray(inputs["adj_cols"]).astype(np.int64)
    adj_vals = np.asarray(inputs["adj_vals"], dtype=np.float32)

    thetas = _theta2()
    W3 = np.asarray(inputs["W3"], dtype=np.float64)
    W3p = np.zeros((3 * H, H), dtype=np.float64)
    for k in range(D + 1):
        for t in range(D + 1):
            W3p[k * H:(k + 1) * H] += thetas[t][k] * W3[t * H:(t + 1) * H]

    idx_wrapped, smat, schedule = _prep_edges(adj_rows, adj_cols, adj_vals)

    nc = _build(schedule)

    in_maps = []
    for c in range(W):
        shard = np.zeros((F_IN, RP), dtype=np.float16)
        shard[:, :R] = in_feat[c * R:(c + 1) * R].T.astype(np.float16)
        in_maps.append({
            "xT": shard,
            "w1": np.asarray(inputs["W1"]).astype(np.float16),
            "w2": np.asarray(inputs["W2"]).astype(np.float16),
            "w3": W3p.astype(np.float16),
            "w4": np.asarray(inputs["W4"]).astype(np.float16),
            "b1": np.asarray(inputs["b1"], dtype=np.float32).reshape(H, 1),
            "b2": np.asarray(inputs["b2"], dtype=np.float32).reshape(H, 1),
            "b3": np.asarray(inputs["b3"], dtype=np.float32).reshape(H, 1),
            "b4": np.asarray(inputs["b4"], dtype=np.float32).reshape(NCLS, 1),
            "idx": idx_wrapped[c],
            "smat": smat[c],
        })
    return nc, in_maps


def kernel(in_feat, adj_rows, adj_cols, adj_vals, W1, b1, W2, b2, W3, b3, W4, b4):
    nc, in_maps = prepare(dict(
        in_feat=in_feat, adj_rows=adj_rows, adj_cols=adj_cols,
        adj_vals=adj_vals, W1=W1, b1=b1, W2=W2, b2=b2, W3=W3, b3=b3,
        W4=W4, b4=b4))
    res = bass_utils.run_bass_kernel_spmd(nc, in_maps, list(range(W)))
    out = np.concatenate(
        [res.results[c]["out"][:, :R].T for c in range(W)], axis=0)
    return np.ascontiguousarray(out, dtype=np.float32)


# revision 3
# speedup vs baseline: 1.0096x; 1.0096x over previous
"""BWGNN (Bernstein-polynomial graph conv, D=2) on 8 Trainium2 NeuronCores. v3.

Algebra as v1/v2 (two SpMMs; thetas folded into W3).  SpMM engine:

- Table is fp16 PAIRS: AllGather output [NPAD, 64] fp16; gather elements are
  two consecutive node rows (256B).  idx = pair index, split in 2 halves of
  25088 (int16 range); gathered data feeds matmul rhs directly (no convert).
- Slot layout: per (block b, half h) a run of cross-core equalized length
  L[b,h] = max_c count, edges sorted by parity (even cols then odd) inside
  the run.  Runs packed half-major per group of GRP blocks; segments padded
  to 128 slots.  One dma_gather per (group, half).
- S matrices are HOST-BUILT one-hots (val at [slot, dest-row]) streamed from
  HBM in piece order, 32 pieces per DMA; piece = chunk x run x parity
  intersection; matmul rhs column-half = parity of the piece.
- PSUM [128, GRP*64] per group; f_next = f - psum per block (DVE subtract).
"""
import math
import numpy as np

import concourse.bass as bass
import concourse.bacc as bacc
import concourse.mybir as mybir
from concourse.tile import TileContext
from concourse.masks import make_identity
from concourse import bass_utils

N = 100000
F_IN = 128
H = 64
NCLS = 2
D = 2
W = 8                   # cores
R = 12500               # real rows per core
RP = 12544              # padded rows per core (98 * 128)
NB = RP // 128          # 98 dest blocks per core
NPAD = W * RP           # 100352 padded table rows
NPAIR = NPAD // 2       # 50176 gather elements (256B each)
NH = 2                  # halves (int16 index range)
HS = NPAIR // NH        # 25088 pairs per half
GRP = 7                 # dest blocks per group (98 = 14 * 7)
NG = NB // GRP
SM_B = 32               # S matrices per streaming tile
F16 = mybir.dt.float16
F32 = mybir.dt.float32
I16 = mybir.dt.int16


def _theta2():
    P = np.polynomial.polynomial
    thetas = []
    for i in range(D + 1):
        beta = math.factorial(i) * math.factorial(D - i) / math.factorial(D + 1)
        c = P.polymul(P.polypow([0.0, 0.5], i), P.polypow([1.0, -0.5], D - i)) / beta
        c = np.pad(c, (0, D + 1 - len(c)))
        thetas.append(c.astype(np.float64))
    return thetas


def _prep_edges(adj_rows, adj_cols, adj_vals):
    """Slot layout with cross-core equalized (block, half) runs, parity-sorted.

    Returns:
      idx_wrapped[c]: [128, S/16] int16 16-wrapped 8x-replicated pair idxs
      smat[c]: [128, npieces*128] fp16 host-built one-hot S matrices
      schedule: static layout (segments, pieces)
    """
    core = adj_rows // R
    rloc = adj_rows - core * R
    blk = rloc // 128
    rowin = rloc % 128
    colp = (adj_cols // R) * RP + (adj_cols % R)     # padded table row
    pair = colp // 2
    par = colp % 2
    half = (pair >= HS).astype(np.int64)
    pidx = pair - half * HS                          # half-local pair index

    # counts per (core, block, half, parity); runs equalized at (b, h) level
    # with the parity boundary ALSO equalized (sort by parity inside run and
    # pad each parity sub-run to its own cross-core max so the boundary is
    # static).
    counts = np.zeros((W, NB, NH, 2), dtype=np.int64)
    np.add.at(counts, (core, blk, half, par), 1)
    Lp = counts.max(axis=0)                          # [NB, NH, 2]

    sub_start = np.zeros((NB, NH, 2), dtype=np.int64)
    seg_start = np.zeros((NG, NH), dtype=np.int64)
    seg_len = np.zeros((NG, NH), dtype=np.int64)
    pos = 0
    for g in range(NG):
        blocks = range(g * GRP, (g + 1) * GRP)
        for hh in range(NH):
            seg_start[g, hh] = pos
            for b in blocks:
                for pp in range(2):
                    sub_start[b, hh, pp] = pos
                    pos += Lp[b, hh, pp]
            pad = -pos % 128
            pos += pad
            seg_len[g, hh] = pos - seg_start[g, hh]
    S_slots = pos
    nchunks = S_slots // 128

    # pieces: (chunk, group, b7, parity, start, stop) in issue order.
    # start/stop are PER BANK (whole psum group tile): matmul start=True
    # clears has_written bits for the entire 2KB zero region, so only the
    # group's first piece starts and its last piece stops; blocks first
    # touched later in the group overwrite via cleared has_written bits.
    pieces = []
    piece_span = []      # (s0, s1) slot range of the piece
    for g in range(NG):
        g_first = len(pieces)
        for hh in range(NH):
            for b in range(g * GRP, (g + 1) * GRP):
                for pp in range(2):
                    s0 = int(sub_start[b, hh, pp])
                    s1 = s0 + int(Lp[b, hh, pp])
                    if s1 == s0:
                        continue
                    for t in range(s0 // 128, (s1 - 1) // 128 + 1):
                        pieces.append([t, g, b - g * GRP, pp, False, False])
                        piece_span.append((max(s0, t * 128),
                                           min(s1, (t + 1) * 128)))
        pieces[g_first][4] = True
        pieces[-1][5] = True
    npieces = len(pieces)

    # per-core slot data
    order = np.lexsort((pidx, par, half, blk, core))
    sc = core[order]
    csel = np.searchsorted(sc, np.arange(W + 1))
    idx_flat = np.zeros((W, S_slots), dtype=np.int16)
    rowv_slot = -np.ones((W, S_slots), dtype=np.int32)
    vals_slot = np.zeros((W, S_slots), dtype=np.float16)
    sb, sh, sp = blk[order], half[order], par[order]
    s_pidx, s_rowin, s_val = pidx[order], rowin[order], adj_vals[order]
    for c in range(W):
        lo, hi = csel[c], csel[c + 1]
        b_arr, h_arr, p_arr = sb[lo:hi], sh[lo:hi], sp[lo:hi]
        key = (b_arr * NH + h_arr) * 2 + p_arr
        brk = np.nonzero(np.diff(key))[0] + 1
        starts = np.concatenate([[0], brk])
        lens = np.diff(np.concatenate([starts, [hi - lo]]))
        pos_in = np.arange(hi - lo) - np.repeat(starts, lens)
        slot = sub_start[b_arr, h_arr, p_arr] + pos_in
        idx_flat[c, slot] = s_pidx[lo:hi].astype(np.int16)
        rowv_slot[c, slot] = s_rowin[lo:hi]
        vals_slot[c, slot] = s_val[lo:hi].astype(np.float16)

    idx_wrapped = []
    for c in range(W):
        a = idx_flat[c].reshape(S_slots // 16, 16).T
        idx_wrapped.append(np.tile(a, (8, 1)).copy())

    # host-built smat [128, npieces*128] fp16 per core
    smat = np.zeros((W, 128, npieces * 128), dtype=np.float16)
    for j, (s0, s1) in enumerate(piece_span):
        t = pieces[j][0]
        p0, p1 = s0 - t * 128, s1 - t * 128
        for c in range(W):
            rv = rowv_slot[c, s0:s1]
            vv = vals_slot[c, s0:s1]
            sel = rv >= 0
            pp = np.nonzero(sel)[0]
            smat[c, p0 + pp, j * 128 + rv[pp]] = vv[pp]

    schedule = dict(S_slots=S_slots, nchunks=nchunks, pieces=pieces,
                    seg_start=seg_start, seg_len=seg_len)
    return idx_wrapped, smat, schedule


def _build(schedule):
    S_slots = schedule["S_slots"]
    pieces = schedule["pieces"]
    seg_start = schedule["seg_start"]
    seg_len = schedule["seg_len"]
    npieces = len(pieces)
    gmax = int(max(sum(seg_len[g]) for g in range(NG))) // 128   # chunks/group

    nc = bacc.Bacc("TRN2", num_swdge_queues=2)
    rg = [list(range(W))]

    xT = nc.dram_tensor("xT", [F_IN, RP], F16, kind="ExternalInput")
    w1 = nc.dram_tensor("w1", [F_IN, H], F16, kind="ExternalInput")
    w2 = nc.dram_tensor("w2", [H, H], F16, kind="ExternalInput")
    w3 = nc.dram_tensor("w3", [3 * H, H], F16, kind="ExternalInput")
    w4 = nc.dram_tensor("w4", [H, NCLS], F16, kind="ExternalInput")
    b1 = nc.dram_tensor("b1", [H, 1], F32, kind="ExternalInput")
    b2 = nc.dram_tensor("b2", [H, 1], F32, kind="ExternalInput")
    b3 = nc.dram_tensor("b3", [H, 1], F32, kind="ExternalInput")
    b4 = nc.dram_tensor("b4", [NCLS, 1], F32, kind="ExternalInput")
    idx_t = nc.dram_tensor("idx", [128, S_slots // 16], I16, kind="ExternalInput")
    smat_t = nc.dram_tensor("smat", [128, npieces * 128], F16,
                            kind="ExternalInput")
    out_t = nc.dram_tensor("out", [NCLS, RP], F32, kind="ExternalOutput")


    ag_in = [nc.dram_tensor(f"agin{i}", [RP, H], F16, kind="Internal")
             for i in range(2)]
    ag_out = [nc.dram_tensor(f"agout{i}", [NPAD, H], F16, kind="Internal",
                             addr_space="Shared") for i in range(2)]

    PCH = 448            # dense-layer column chunk (28 * 448 = 12544)

    with TileContext(nc) as tc:
        with tc.tile_pool(name="c0", bufs=1) as cpool, \
             tc.tile_pool(name="mm", bufs=3) as mpool, \
             tc.tile_pool(name="gg", bufs=2) as gpool, \
             tc.tile_pool(name="ss", bufs=3) as spool, \
             tc.tile_pool(name="ps", bufs=2, space="PSUM") as pspool, \
             tc.tile_pool(name="pb", bufs=2, space="PSUM") as pbpool, \
             tc.tile_pool(name="pg", bufs=2, space="PSUM") as pgpool:

            ident = cpool.tile([128, 128], F16)
            make_identity(nc, ident[:])

            def load_const(name, src, shape, dt):
                tile = cpool.tile(shape, dt, tag=name)
                nc.sync.dma_start(out=tile[:], in_=src)
                return tile

            w1_sb = load_const("w1", w1[:], [F_IN, H], F16)
            w2_sb = load_const("w2", w2[:], [H, H], F16)
            w3ab_sb = load_const("w3ab", w3[0:128, :], [128, H], F16)
            w3c_sb = load_const("w3c", w3[128:192, :], [H, H], F16)
            w4_sb = load_const("w4", w4[:], [H, NCLS], F16)
            b1_sb = load_const("b1", b1[:], [H, 1], F32)
            b2_sb = load_const("b2", b2[:], [H, 1], F32)
            b3_sb = load_const("b3", b3[:], [H, 1], F32)
            b4_sb = load_const("b4", b4[:], [NCLS, 1], F32)
            idx_sb = load_const("idx", idx_t[:], [128, S_slots // 16], I16)

            h1_f2 = cpool.tile([128, RP], F16)   # h1 then feat2 (fm) on p0..63
            h_cat = cpool.tile([128, RP], F16)   # feat0 p0..63, feat1 p64..127
            f0_rm = cpool.tile([128, NB * H], F16)
            f1_rm = cpool.tile([128, NB * H], F16)
            f2_rm = f0_rm     # feat0_rm dead once SpMM1's subtract ran

            # ---------- MLP1 + MLP2 (feature-major fp16) ----------
            for o in range(0, RP, PCH):
                xt = mpool.tile([F_IN, PCH], F16, tag="xin")
                nc.sync.dma_start(out=xt[:], in_=xT[:, o:o + PCH])
                pt = pspool.tile([H, PCH], F32, tag="pmlp", space="PSUM")
                nc.tensor.matmul(pt[:], lhsT=w1_sb[:], rhs=xt[:],
                                 start=True, stop=True)
                nc.scalar.activation(h1_f2[0:H, o:o + PCH], pt[:],
                                     mybir.ActivationFunctionType.Relu,
                                     bias=b1_sb[:], scale=1.0)
            for o in range(0, RP, PCH):
                pt = pspool.tile([H, PCH], F32, tag="pmlp", space="PSUM")
                nc.tensor.matmul(pt[:], lhsT=w2_sb[:], rhs=h1_f2[0:H, o:o + PCH],
                                 start=True, stop=True)
                nc.scalar.activation(h_cat[0:H, o:o + PCH], pt[:],
                                     mybir.ActivationFunctionType.Relu,
                                     bias=b2_sb[:], scale=1.0)

            # ---------- feat0 -> row-major, ship to AllGather ----------
            for b in range(NB):
                pt = pbpool.tile([128, 128], F16, tag="ptr", space="PSUM")
                nc.tensor.transpose(pt[0:128, 0:H],
                                    h_cat[0:H, b * 128:(b + 1) * 128],
                                    ident[0:H, 0:H])
                nc.vector.tensor_copy(f0_rm[:, b * H:(b + 1) * H], pt[0:128, 0:H])
            nc.sync.dma_start(
                out=ag_in[0][:].rearrange("(t p) h -> p t h", p=128),
                in_=f0_rm[:].rearrange("p (t h) -> p t h", h=H))
            nc.gpsimd.collective_compute(
                "AllGather", mybir.AluOpType.bypass, replica_groups=rg,
                ins=[ag_in[0][:]], outs=[ag_out[0][:]])

            # ---------- SpMM pass ----------
            def spmm(src, cur_rm, nxt_rm, ag_next):
                src_flat = src[:].rearrange("n h -> (n h)")
                hviews = [src_flat[hh * HS * 128:(hh + 1) * HS * 128].rearrange(
                    "(q s) -> q s", s=128) for hh in range(NH)]
                pc = 0
                for g in range(NG):
                    g16 = gpool.tile([128, gmax * 128], F16, tag="g16")
                    goff = 0
                    for hh in range(NH):
                        s0 = int(seg_start[g, hh])
                        ln = int(seg_len[g, hh])
                        lc = ln // 128
                        nc.gpsimd.dma_gather(
                            out_ap=g16[:, goff * 128:(goff + lc) * 128]
                            .rearrange("p (t e) -> p t e", e=128),
                            in_ap=hviews[hh],
                            idxs_ap=idx_sb[:, s0 // 16:(s0 + ln) // 16],
                            num_idxs=ln,
                            num_idxs_reg=ln,
                            elem_size=128,
                            single_packet=False,
                            queue_num=hh,
                        )
                        goff += lc
                    gbase = int(seg_start[g, 0]) // 128
                    psum_g = pgpool.tile([128, GRP * H], F32, tag="pgrp",
                                         space="PSUM")
                    while pc < npieces and pieces[pc][1] == g:
                        # stream SM_B pieces of smat at a time
                        j0 = pc
                        j1 = min(j0 + SM_B, npieces)
                        while j1 > j0 and pieces[j1 - 1][1] != g:
                            j1 -= 1
                        sm = spool.tile([128, SM_B * 128], F16, tag="sm")
                        nc.sync.dma_start(
                            out=sm[:, 0:(j1 - j0) * 128],
                            in_=smat_t[:, j0 * 128:j1 * 128])
                        for j in range(j0, j1):
                            t, _, b7, pp, st, sp = pieces[j]
                            tl = t - gbase
                            nc.tensor.matmul(
                                psum_g[:, b7 * H:(b7 + 1) * H],
                                lhsT=sm[:, (j - j0) * 128:(j - j0 + 1) * 128],
                                rhs=g16[:, tl * 128 + pp * H:
                                        tl * 128 + (pp + 1) * H],
                                start=st, stop=sp)
                        pc = j1
                    for b7 in range(GRP):
                        b = g * GRP + b7
                        nc.vector.tensor_tensor(
                            out=nxt_rm[:, b * H:(b + 1) * H],
                            in0=cur_rm[:, b * H:(b + 1) * H],
                            in1=psum_g[:, b7 * H:(b7 + 1) * H],
                            op=mybir.AluOpType.subtract)
                if ag_next is not None:
                    nc.sync.dma_start(
                        out=ag_next[:].rearrange("(t p) h -> p t h", p=128),
                        in_=nxt_rm[:].rearrange("p (t h) -> p t h", h=H))

            spmm(ag_out[0], f0_rm, f1_rm, ag_in[1])
            nc.gpsimd.collective_compute(
                "AllGather", mybir.AluOpType.bypass, replica_groups=rg,
                ins=[ag_in[1][:]], outs=[ag_out[1][:]])

            # f1 -> feature-major while AllGather of f1 runs
            for b in range(NB):
                pt = pbpool.tile([128, 128], F16, tag="ptr", space="PSUM")
                nc.tensor.transpose(pt[0:H, 0:128], f1_rm[:, b * H:(b + 1) * H],
                                    ident[:])
                nc.vector.tensor_copy(h_cat[H:128, b * 128:(b + 1) * 128],
                                      pt[0:H, 0:128])

            spmm(ag_out[1], f1_rm, f2_rm, None)

            # ---------- feat2 back to feature-major ----------
            for b in range(NB):
                pt = pbpool.tile([128, 128], F16, tag="ptr", space="PSUM")
                nc.tensor.transpose(pt[0:H, 0:128], f2_rm[:, b * H:(b + 1) * H],
                                    ident[:])
                nc.vector.tensor_copy(h1_f2[0:H, b * 128:(b + 1) * 128],
                                      pt[0:H, 0:128])

            # ---------- MLP3 + MLP4 fused ----------
            for o in range(0, RP, PCH):
                pt = pspool.tile([H, PCH], F32, tag="pmlp", space="PSUM")
                nc.tensor.matmul(pt[:], lhsT=w3ab_sb[:], rhs=h_cat[:, o:o + PCH],
                                 start=True, stop=False)
                nc.tensor.matmul(pt[:], lhsT=w3c_sb[:], rhs=h1_f2[0:H, o:o + PCH],
                                 start=False, stop=True)
                h3 = mpool.tile([H, PCH], F16, tag="h3")
                nc.scalar.activation(h3[:], pt[:],
                                     mybir.ActivationFunctionType.Relu,
                                     bias=b3_sb[:], scale=1.0)
                po = pspool.tile([NCLS, PCH], F32, tag="pout", space="PSUM")
                nc.tensor.matmul(po[:], lhsT=w4_sb[:], rhs=h3[:],
                                 start=True, stop=True)
                ot = mpool.tile([NCLS, PCH], F32, tag="ot")
                nc.scalar.activation(ot[:], po[:],
                                     mybir.ActivationFunctionType.Identity,
                                     bias=b4_sb[:], scale=1.0)
                nc.sync.dma_start(out=out_t[:, o:o + PCH], in_=ot[:])

    nc.compile()
    return nc


def prepare(inputs):
    """Build (nc, in_maps) for the full input dict."""
    in_feat = np.asarray(inputs["in_feat"], dtype=np.float32)
    adj_rows = np.asarray(inputs["adj_rows"]).astype(np.int64)
    adj_cols = np.asarray(inputs["adj_cols"]).astype(np.int64)
    adj_vals = np.asarray(inputs["adj_vals"], dtype=np.float32)

    thetas = _theta2()
    W3 = np.asarray(inputs["W3"], dtype=np.float64)
    W3p = np.zeros((3 * H, H), dtype=np.float64)
    for k in range(D + 1):
        for t in range(D + 1):
            W3p[k * H:(k + 1) * H] += thetas[t][k] * W3[t * H:(t + 1) * H]

    idx_wrapped, smat, schedule = _prep_edges(adj_rows, adj_cols, adj_vals)

    nc = _build(schedule)

    in_maps = []
    for c in range(W):
        shard = np.zeros((F_IN, RP), dtype=np.float16)
        shard[:, :R] = in_feat[c * R:(c + 1) * R].T.astype(np.float16)
        in_maps.append({
            "xT": shard,
            "w1": np.asarray(inputs["W1"]).astype(np.float16),
            "w2": np.asarray(inputs["W2"]).astype(np.float16),
            "w3": W3p.astype(np.float16),
            "w4": np.asarray(inputs["W4"]).astype(np.float16),
            "b1": np.asarray(inputs["b1"], dtype=np.float32).reshape(H, 1),
            "b2": np.asarray(inputs["b2"], dtype=np.float32).reshape(H, 1),
            "b3": np.asarray(inputs["b3"], dtype=np.float32).reshape(H, 1),
            "b4": np.asarray(inputs["b4"], dtype=np.float32).reshape(NCLS, 1),
            "idx": idx_wrapped[c],
            "smat": smat[c],
        })
    return nc, in_maps


def kernel(in_feat, adj_rows, adj_cols, adj_vals, W1, b1, W2, b2, W3, b3, W4, b4):
    nc, in_maps = prepare(dict(
        in_feat=in_feat, adj_rows=adj_rows, adj_cols=adj_cols,
        adj_vals=adj_vals, W1=W1, b1=b1, W2=W2, b2=b2, W3=W3, b3=b3,
        W4=W4, b4=b4))
    res = bass_utils.run_bass_kernel_spmd(nc, in_maps, list(range(W)))
    out = np.concatenate(
        [res.results[c]["out"][:, :R].T for c in range(W)], axis=0)
    return np.ascontiguousarray(out, dtype=np.float32)
